# revision 10
# baseline (speedup 1.0000x reference)
"""Trainium2 Bass kernel for nn_MultiHeadedAttention_44624710205499.

Reference computation (B=4, S=2048, D=512, H=8, dk=64, L=5):
  q = local_pool(query, 5)                    # causal 5-window softmax pooling
  k = local_pool(key @ W_fk + b_fk, 5)
  v = value @ W0 + b0
  x = MHA(q, k, v)   (full softmax, no mask)
  out = x @ Wout + bout

Sharding: 8 cores = (batch b = c//2) x (query-half = c%2).  Each core
computes 1024 query rows of one batch against all 2048 keys of that batch.
Gather on host is pure concatenation (+ transpose of the core's [D, SQ]
output layout).

On-device layout strategy: "feature-on-partition" (transposed) layout
throughout, which makes every step a natural matmul with no on-device
transposes:
  - scoresT[k, q] = kT-chunk.T @ qT          (lhsT=kT slice, rhs=qT slice)
  - eT = exp(scoresT)  (no max-subtraction: scores bounded ~30, fp32 safe)
  - x_augT[dv+1, q] = v_aug.T @ eT  accumulated over key chunks, where
    v_aug has a ones column => softmax denominator falls out as row 64.
  - divide by denominator after PV (softmax normalization is linear-safe)
  - outT = Wout-chunk.T @ xT  (host transposes the final [D, SQ] output)
Local pooling is banded attention over 516-row context windows with a
constant band mask; front zero-padding reproduces the reference's
zero-vector padding semantics exactly (score 0 -> weight exp(0)=1 in the
denominator, zero contribution to the numerator).

Matmul dtypes: float32r (1 cyc/row at N>=512 vs 4 for fp32) for all big
matmuls; walrus requires every producer of an f32r matmul operand to write
an f32r-declared tensor, so those tiles/DRAM tensors are declared float32r
(byte-identical to fp32 on the host side).  The pooling score matmuls use
bf16 operands: pooling is self-dominant (self score ~22.6 vs ~1 for
neighbors), so score-side quantization error cancels between numerator and
denominator, while the value-side path (kf_row/qrow) stays full precision.
"""

import math
import os

import ml_dtypes
import numpy as np

import concourse.bass as bass
import concourse.tile as tile
from concourse import bacc, mybir
from concourse import bass_utils

P = 128
B, S, D, H, DK, L = 4, 2048, 512, 8, 64, 5
SQ = S // 2            # query rows per core
NKI = D // P           # 4 contraction chunks of 128
SPAD = S + (L - 1)     # 2052 zero-front-padded kf length
SQPAD = SQ + (L - 1)   # 1028 query halo length
BLK = 512              # pooling block (positions per block)
NCH = 5                # ctx chunks per pooling block: 4x128 + 4
NBK = S // BLK         # 4 kf pooling blocks
NBQ = SQ // BLK        # 2 q pooling blocks
NQC = SQ // BLK        # 2 SDPA query chunks of 512
NKC = S // P           # 16 SDPA key chunks of 128
RSQD = 1.0 / math.sqrt(D)
RSQK = 1.0 / math.sqrt(DK)
NCORES = 8

F32 = mybir.dt.float32
BF16 = mybir.dt.bfloat16
F32R = mybir.dt.float32r

_PROG_CACHE = {}


def build_program(cfg=None):
    """Build + compile the per-core Bass program (same program on all 8 cores)."""
    cfg = dict(cfg or {})
    MDT = F32R if cfg.get("use_f32r", True) else F32   # big-matmul dtype
    sc_dt = BF16 if cfg.get("bf16_scores", True) else MDT

    def rd(ap):
        """read-view of an MDT tile for non-matmul (DVE/ACT) consumers."""
        return ap.bitcast(F32) if MDT == F32R else ap

    nc = bacc.Bacc(
        "TRN2",
        target_bir_lowering=False,
        debug=False,
        enable_asserts=False,
        num_devices=NCORES,
    )

    keyT_d = nc.dram_tensor("keyT", [D, SPAD], MDT, kind="ExternalInput").ap()
    valT_d = nc.dram_tensor("valT", [D, S], MDT, kind="ExternalInput").ap()
    qT_d = nc.dram_tensor("qT", [D, SQPAD], sc_dt, kind="ExternalInput").ap()
    qrow_d = nc.dram_tensor("qrow", [SQPAD, D], MDT, kind="ExternalInput").ap()
    wfk_d = nc.dram_tensor("wfk", [D, D], MDT, kind="ExternalInput").ap()
    w0_d = nc.dram_tensor("w0", [D, D], MDT, kind="ExternalInput").ap()
    wout_d = nc.dram_tensor("wout", [D, D], MDT, kind="ExternalInput").ap()
    ones_d = nc.dram_tensor("ones_col", [P, 1], MDT, kind="ExternalInput").ap()
    vones_d = nc.dram_tensor("vones", [P, 2, 8 * H], MDT, kind="ExternalInput").ap()
    bfk_col_d = nc.dram_tensor("bfk_col", [D, 1], F32, kind="ExternalInput").ap()
    bfk_row_d = nc.dram_tensor("bfk_row", [1, D], F32, kind="ExternalInput").ap()
    b0_row_d = nc.dram_tensor("b0_row", [1, D], F32, kind="ExternalInput").ap()
    bout_col_d = nc.dram_tensor("bout_col", [D, 1], F32, kind="ExternalInput").ap()
    mask_d = nc.dram_tensor("mask_band", [NCH * P, BLK], F32, kind="ExternalInput").ap()
    kfpad_d = nc.dram_tensor("kfpad", [D, L - 1], sc_dt, kind="ExternalInput").ap()
    outT_d = nc.dram_tensor("outT", [D, SQ], F32, kind="ExternalOutput").ap()
    if cfg.get("dbg"):
        dbg_kTp_d = nc.dram_tensor("dbg_kTp", [D, S], F32, kind="ExternalOutput").ap()
        dbg_qTp_d = nc.dram_tensor("dbg_qTp", [D, SQ], F32, kind="ExternalOutput").ap()
        dbg_xt_d = nc.dram_tensor("dbg_xt", [D, SQ], F32, kind="ExternalOutput").ap()
        dbg_rec_d = nc.dram_tensor("dbg_rec", [H, BLK], F32, kind="ExternalOutput").ap()
        dbg_kf_d = nc.dram_tensor("dbg_kf", [D, S], sc_dt, kind="ExternalOutput").ap()

    with tile.TileContext(nc) as tc:
        with (
            tc.tile_pool(name="A", bufs=4) as pA,      # keyT -> valT -> outT
            tc.tile_pool(name="Bp", bufs=4) as pB,     # kfT -> xt4
            tc.tile_pool(name="C", bufs=2) as pC,      # kfr -> qT/qrow -> v
            tc.tile_pool(name="W", bufs=2) as pW,      # wfk -> w0 -> wout
            tc.tile_pool(name="kTp", bufs=4) as pK,
            tc.tile_pool(name="qTp", bufs=4) as pQ,
            tc.tile_pool(name="small", bufs=1) as pS,
            tc.tile_pool(name="esc", bufs=4) as pE,    # pooling exp tiles
            tc.tile_pool(name="esb", bufs=4) as pEb,   # SDPA exp tiles
            tc.tile_pool(name="rec", bufs=6) as pR,    # recips/broadcasts/tmp
            tc.tile_pool(name="psS", bufs=3, space="PSUM") as psS,
            tc.tile_pool(name="psV", bufs=4, space="PSUM") as psV,
            tc.tile_pool(name="psD", bufs=1, space="PSUM") as psD,
        ):
            # ---------------- constants / small loads ----------------
            mask_sb = pS.tile([P, NCH, BLK], F32, tag="mask")
            nc.sync.dma_start(mask_sb[:], mask_d.rearrange("(m p) i -> p m i", p=P))
            bfk_col = pS.tile([P, NKI, 1], F32, tag="bfkc")
            nc.sync.dma_start(bfk_col[:], bfk_col_d.rearrange("(k p) o -> p k o", p=P))
            bout_col = pS.tile([P, NKI, 1], F32, tag="boutc")
            nc.sync.dma_start(bout_col[:], bout_col_d.rearrange("(k p) o -> p k o", p=P))
            ones_sb = pS.tile([P, 1], MDT, tag="ones")
            nc.sync.dma_start(ones_sb[:], ones_d[:])

            bfk_row = pR.tile([1, D], F32, tag="rec")
            nc.sync.dma_start(bfk_row[:], bfk_row_d[:])
            bfk_bc = pR.tile([P, D], F32, tag="rec")
            nc.gpsimd.partition_broadcast(bfk_bc[:], bfk_row[:])
            # variant with the 4 pad rows zeroed (for kf_row tile 0)
            bfk_bc0 = pS.tile([P, D], F32, tag="bfkbc0")
            nc.gpsimd.partition_broadcast(bfk_bc0[:], bfk_row[:])
            nc.vector.memset(bfk_bc0[0 : L - 1, :], 0.0)

            # ---------------- keyT + wfk loads ----------------
            keyT = [pA.tile([P, SPAD], MDT, tag="A", name=f"keyT{t}") for t in range(NKI)]
            for t in range(NKI):
                nc.sync.dma_start(keyT[t][:], keyT_d[P * t : P * (t + 1), :])
            wfk = pW.tile([P, NKI, D], MDT, tag="W")
            nc.sync.dma_start(wfk[:], wfk_d.rearrange("(k p) n -> p k n", p=P))

            # ---------------- kfT = (key @ W_fk + b_fk).T  [D, SPAD] ----------
            kfT = [pB.tile([P, SPAD], sc_dt, tag="B", name=f"kfT{t}") for t in range(NKI)]
            for mo in range(NKI):
                nc.sync.dma_start(kfT[mo][:, 0 : L - 1], kfpad_d[P * mo : P * (mo + 1), :])
            for mo in range(NKI):
                for ns in range(S // BLK):
                    ps = psS.tile([P, BLK], F32, tag="psS")
                    for ki in range(NKI):
                        nc.tensor.matmul(
                            ps[:],
                            wfk[:, ki, P * mo : P * (mo + 1)],
                            keyT[ki][:, L - 1 + BLK * ns : L - 1 + BLK * (ns + 1)],
                            start=(ki == 0),
                            stop=(ki == NKI - 1),
                        )
                    nc.vector.tensor_scalar_add(
                        kfT[mo][:, L - 1 + BLK * ns : L - 1 + BLK * (ns + 1)],
                        ps[:],
                        bfk_col[:, mo, :],
                    )

            # ---------------- kf_row  [SPAD rows, D]  (17 x 128-row tiles) -----
            kfrA = pC.tile([P, 9, BLK], MDT, tag="C")
            kfrB = pC.tile([P, 8, BLK], MDT, tag="C")

            def kfr(n):
                return (kfrA, n) if n < 9 else (kfrB, n - 9)

            NROW = SPAD // P + 1  # 17
            for n in range(NROW):
                M = P if n < NROW - 1 else SPAD - P * (NROW - 1)  # 128 or 4
                ps = psS.tile([P, BLK], F32, tag="psS")
                for ki in range(NKI):
                    nc.tensor.matmul(
                        ps[0:M, :],
                        keyT[ki][:, P * n : P * n + M],
                        wfk[:, ki, :],
                        start=(ki == 0),
                        stop=(ki == NKI - 1),
                    )
                t_, idx = kfr(n)
                # tile 0 partitions 0:4 are the zero pad rows: psum rows are 0
                # there (zero key columns) and bfk_bc0 keeps them 0.
                bias = bfk_bc0 if n == 0 else bfk_bc
                nc.vector.tensor_add(t_[0:M, idx, :], ps[0:M, :], bias[0:M, :])

            # ---------------- pooling (banded attention over 516-row ctx) -----
            def emit_pool(xT_slice, xrow, out_tiles, nblocks):
                """xT_slice(ki) -> [P, *PAD] transposed (padded) AP, sc_dt.
                xrow(n) -> (tile, idx) row-layout 128-row chunk n (padded rows).
                out_tiles: 4 x [P, nblocks*BLK] MDT pooled output (transposed)."""
                for t in range(nblocks):
                    es = []
                    for m in range(NCH):
                        K = P if m < NCH - 1 else L - 1
                        ps = psS.tile([P, BLK], F32, tag="psS")
                        for ki in range(NKI):
                            xa = xT_slice(ki)
                            nc.tensor.matmul(
                                ps[0:K, :],
                                xa[:, BLK * t + P * m : BLK * t + P * m + K],
                                xa[:, L - 1 + BLK * t : L - 1 + BLK * (t + 1)],
                                start=(ki == 0),
                                stop=(ki == NKI - 1),
                            )
                        e = pE.tile([P, BLK], MDT, tag="esc")
                        nc.scalar.activation(
                            e[0:K, :], ps[0:K, :],
                            mybir.ActivationFunctionType.Exp, scale=RSQD,
                        )
                        nc.vector.tensor_mul(e[0:K, :], rd(e[0:K, :]), mask_sb[0:K, m, :])
                        es.append(e)
                    dn = psD.tile([1, BLK], F32, tag="psD")
                    for m in range(NCH):
                        K = P if m < NCH - 1 else L - 1
                        nc.tensor.matmul(
                            dn[:],
                            ones_sb[0:K, :],
                            es[m][0:K, :],
                            start=(m == 0),
                            stop=(m == NCH - 1),
                        )
                    rc = pR.tile([1, BLK], F32, tag="rec")
                    nc.vector.reciprocal(rc[:], dn[:])
                    rb = pR.tile([P, BLK], F32, tag="rec")
                    nc.gpsimd.partition_broadcast(rb[:], rc[:])
                    pvs = [psV.tile([P, BLK], F32, tag="psV", name=f"pv{mo}")
                           for mo in range(NKI)]
                    for mo in range(NKI):
                        for m in range(NCH):
                            K = P if m < NCH - 1 else L - 1
                            t_, idx = xrow(4 * t + m)
                            nc.tensor.matmul(
                                pvs[mo][:],
                                t_[0:K, idx, P * mo : P * (mo + 1)],
                                es[m][0:K, :],
                                start=(m == 0),
                                stop=(m == NCH - 1),
                            )
                    for mo in range(NKI):
                        nc.vector.tensor_mul(
                            out_tiles[mo][:, BLK * t : BLK * (t + 1)], pvs[mo][:], rb[:]
                        )

            kTp = [pK.tile([P, S], MDT, tag="kTp", name=f"kTp{t}") for t in range(NKI)]
            emit_pool(lambda ki: kfT[ki][:], kfr, kTp, NBK)

            # ---------------- q loads + q pooling ----------------
            qT_all = pC.tile([P, NKI, SQPAD], sc_dt, tag="C")
            nc.sync.dma_start(qT_all[:], qT_d.rearrange("(t p) s -> p t s", p=P))
            qrowA = pC.tile([P, 9, BLK], MDT, tag="C")
            nc.sync.dma_start(
                qrowA[:, 0:8, :], qrow_d[0:SQ, :].rearrange("(n p) d -> p n d", p=P)
            )
            nc.sync.dma_start(qrowA[0:4, 8, :], qrow_d[SQ:SQPAD, :])

            qTp = [pQ.tile([P, SQ], MDT, tag="qTp", name=f"qTp{t}") for t in range(NKI)]
            emit_pool(lambda ki: qT_all[:, ki, :], lambda n: (qrowA, n), qTp, NBQ)

            # ---------------- v = value @ W0 + b0   [S rows, H, 65] -----------
            w0 = pW.tile([P, NKI, D], MDT, tag="W")
            nc.sync.dma_start(w0[:], w0_d.rearrange("(k p) n -> p k n", p=P))
            valT = [pA.tile([P, S], MDT, tag="A", name=f"valT{t}") for t in range(NKI)]
            for t in range(NKI):
                nc.sync.dma_start(valT[t][:], valT_d[P * t : P * (t + 1), :])
            b0_row = pR.tile([1, D], F32, tag="rec")
            nc.sync.dma_start(b0_row[:], b0_row_d[:])
            b0_bc = pR.tile([P, D], F32, tag="rec")
            nc.gpsimd.partition_broadcast(b0_bc[:], b0_row[:])

            vA = pC.tile([P, 8, H, DK + 1], MDT, tag="C")
            vB = pC.tile([P, 8, H, DK + 1], MDT, tag="C")
            nc.sync.dma_start(vA[:, :, :, DK], vones_d[:, 0, :].rearrange("p (n h) -> p n h", n=8))
            nc.sync.dma_start(vB[:, :, :, DK], vones_d[:, 1, :].rearrange("p (n h) -> p n h", n=8))
            for n in range(NKC):
                ps = psS.tile([P, BLK], F32, tag="psS")
                for ki in range(NKI):
                    nc.tensor.matmul(
                        ps[:],
                        valT[ki][:, P * n : P * (n + 1)],
                        w0[:, ki, :],
                        start=(ki == 0),
                        stop=(ki == NKI - 1),
                    )
                vt = vA if n < 8 else vB
                nc.vector.tensor_add(
                    vt[:, n % 8, :, 0:DK],
                    ps[:].rearrange("p (h z) -> p h z", h=H),
                    b0_bc[:].rearrange("p (h z) -> p h z", h=H),
                )

            # ---------------- SDPA + output projection ----------------
            wout = pW.tile([P, NKI, D], MDT, tag="W")
            nc.sync.dma_start(wout[:], wout_d.rearrange("(k p) n -> p k n", p=P))
            xt4 = [pB.tile([P, SQ], MDT, tag="B", name=f"xt4_{t}") for t in range(NKI)]
            outT = [pA.tile([P, SQ], F32, tag="A", name=f"outT{t}") for t in range(NKI)]

            for qc in range(NQC):
                for h in range(H):
                    th, off = h // 2, DK * (h % 2)
                    px = psV.tile([DK + 1, BLK], F32, tag="psV")
                    for kc in range(NKC):
                        ps = psS.tile([P, BLK], F32, tag="psS")
                        nc.tensor.matmul(
                            ps[:],
                            kTp[th][off : off + DK, P * kc : P * (kc + 1)],
                            qTp[th][off : off + DK, BLK * qc : BLK * (qc + 1)],
                            start=True,
                            stop=True,
                        )
                        e = pEb.tile([P, BLK], MDT, tag="esb")
                        nc.scalar.activation(
                            e[:], ps[:], mybir.ActivationFunctionType.Exp, scale=RSQK
                        )
                        vt = vA if kc < 8 else vB
                        nc.tensor.matmul(
                            px[:],
                            vt[:, kc % 8, h, :],
                            e[:],
                            start=(kc == 0),
                            stop=(kc == NKC - 1),
                        )
                    # partition_broadcast HW ucode reads tile partition 0,
                    # not the AP base -- reciprocal at base 64 (lane-aligned),
                    # then DMA the row down to a base-0 tile for the broadcast.
                    rc = pR.tile([DK + 1, BLK], F32, tag="rec")
                    nc.vector.reciprocal(rc[DK : DK + 1, :], px[DK : DK + 1, :])
                    rc0 = pR.tile([1, BLK], F32, tag="rec")
                    nc.sync.dma_start(rc0[:], rc[DK : DK + 1, :])
                    rb = pR.tile([DK, BLK], F32, tag="rec")
                    nc.gpsimd.partition_broadcast(rb[:], rc0[:])
                    if cfg.get("dbg") and qc == 0:
                        nc.sync.dma_start(dbg_rec_d[h : h + 1, :], rb[0:1, :])
                    if h % 2 == 0:
                        nc.vector.tensor_mul(
                            xt4[th][0:DK, BLK * qc : BLK * (qc + 1)], px[0:DK, :], rb[:]
                        )
                    else:
                        tmp = pR.tile([DK, BLK], MDT, tag="rectmp")
                        nc.vector.tensor_mul(tmp[:], px[0:DK, :], rb[:])
                        nc.sync.dma_start(
                            xt4[th][DK:P, BLK * qc : BLK * (qc + 1)], tmp[:]
                        )
                for mo in range(NKI):
                    po = psS.tile([P, BLK], F32, tag="psS")
                    for ki in range(NKI):
                        nc.tensor.matmul(
                            po[:],
                            wout[:, ki, P * mo : P * (mo + 1)],
                            xt4[ki][:, BLK * qc : BLK * (qc + 1)],
                            start=(ki == 0),
                            stop=(ki == NKI - 1),
                        )
                    nc.vector.tensor_scalar_add(
                        outT[mo][:, BLK * qc : BLK * (qc + 1)], po[:], bout_col[:, mo, :]
                    )
            for mo in range(NKI):
                nc.sync.dma_start(outT_d[P * mo : P * (mo + 1), :], outT[mo][:])
            if cfg.get("dbg"):
                for t in range(NKI):
                    nc.sync.dma_start(dbg_kTp_d[P * t : P * (t + 1), :],
                                      rd(kTp[t][:]))
                    nc.sync.dma_start(dbg_qTp_d[P * t : P * (t + 1), :],
                                      rd(qTp[t][:]))
                    nc.sync.dma_start(dbg_xt_d[P * t : P * (t + 1), :],
                                      rd(xt4[t][:]))
                    nc.sync.dma_start(dbg_kf_d[P * t : P * (t + 1), :],
                                      kfT[t][:, L - 1 :])

    nc.compile()
    return nc


def make_band_mask():
    j = np.arange(NCH * P)[:, None]
    i = np.arange(BLK)[None, :]
    return (((j - i) >= 0) & ((j - i) <= L - 1)).astype(np.float32)


def make_core_inputs(query, key, value, W_fk, b_fk, W0, b0, Wout, bout, cfg=None):
    """Build the 8 per-core input dicts from full inputs (host-side shard)."""
    cfg = dict(cfg or {})
    bf16_scores = cfg.get("bf16_scores", True)
    sc_np = ml_dtypes.bfloat16 if bf16_scores else np.float32
    shared = {
        "wfk": np.ascontiguousarray(W_fk, np.float32),
        "w0": np.ascontiguousarray(W0, np.float32),
        "wout": np.ascontiguousarray(Wout, np.float32),
        "ones_col": np.ones((P, 1), np.float32),
        "vones": np.ones((P, 2, 8 * H), np.float32),
        "bfk_col": np.ascontiguousarray(b_fk.reshape(D, 1), np.float32),
        "bfk_row": np.ascontiguousarray(b_fk.reshape(1, D), np.float32),
        "b0_row": np.ascontiguousarray(b0.reshape(1, D), np.float32),
        "bout_col": np.ascontiguousarray(bout.reshape(D, 1), np.float32),
        "mask_band": make_band_mask(),
        "kfpad": np.zeros((D, L - 1), sc_np),
    }
    in_maps = []
    for c in range(NCORES):
        b, half = divmod(c, 2)
        q0 = half * SQ
        q_halo = np.zeros((SQPAD, D), np.float32)
        lo = max(0, q0 - (L - 1))
        q_halo[(L - 1) - (q0 - lo):] = query[b, lo : q0 + SQ]
        keyT_pad = np.zeros((D, SPAD), np.float32)
        keyT_pad[:, L - 1 :] = key[b].T
        m = dict(shared)
        m["keyT"] = keyT_pad
        m["valT"] = np.ascontiguousarray(value[b].T, np.float32)
        m["qT"] = np.ascontiguousarray(q_halo.T).astype(sc_np)
        m["qrow"] = q_halo
        in_maps.append(m)
    return in_maps


def _cfg_from_env():
    cfg_key = os.environ.get("ATT_KERNEL_CFG", "")
    cfg = {}
    if "no_f32r" in cfg_key:
        cfg["use_f32r"] = False
    if "f32_scores" in cfg_key:
        cfg["bf16_scores"] = False
    return cfg


def get_program(cfg=None):
    cfg = dict(cfg or {})
    key_t = tuple(sorted(cfg.items()))
    if key_t not in _PROG_CACHE:
        _PROG_CACHE[key_t] = build_program(cfg)
    return _PROG_CACHE[key_t]


def kernel(query, key, value, mask=None, W_fk=None, b_fk=None, W0=None, b0=None,
           Wout=None, bout=None, **extra):
    del mask, extra  # mask is dead in the reference (forward passes mask=None)
    cfg = _cfg_from_env()
    nc = get_program(cfg)

    query = np.asarray(query, np.float32)
    key = np.asarray(key, np.float32)
    value = np.asarray(value, np.float32)
    in_maps = make_core_inputs(
        query, key, value,
        np.asarray(W_fk, np.float32), np.asarray(b_fk, np.float32),
        np.asarray(W0, np.float32), np.asarray(b0, np.float32),
        np.asarray(Wout, np.float32), np.asarray(bout, np.float32),
        cfg,
    )
    res = bass_utils.run_bass_kernel_spmd(nc, in_maps, core_ids=list(range(NCORES)))
    out = np.empty((B, S, D), np.float32)
    for c in range(NCORES):
        b, half = divmod(c, 2)
        out[b, half * SQ : (half + 1) * SQ, :] = res.results[c]["outT"].T
    return out
